# revision 56
# baseline (speedup 1.0000x reference)
"""Trainium2 Bass kernel for nn_AttentionLayerDecoder (sparse segment attention).

Math (reference, with edge_index unused):
  query[h,b,v] = context[b,:] @ Wq[h]                      # [H,B,Dv]
  u[h,n]      = (x[n,:] @ Wk[h]) . query[h,batch[n],:] / sqrt(Dv)
  a[h,n]      = segment_softmax(u) over nodes of graph batch[n]
  agg[h,b,v]  = sum_{n in b} a[h,n] * (x[n,:] @ Wv[h])
  out[b,:]    = sum_h (qc*query[h,b,:] + agg[h,b,:]) @ Wf

Device computes, per graph g (sharded 16 whole graphs per core):
  u[n,h]  = x[n,:] . qk[g,h,:]        (PE, lhsT = x_T tile, rhs = qk)
  e[n,h]  = exp(u - 1)                (ACT, batched 4 graphs, fp8 out;
                                       -1 bias keeps e in fp8e4m3 range)
  xe[h,:] = sum_n e[n,h] * [x[n,:],1] (PE fp8 DoubleRow, ones column
                                       makes col 128 = S = sum_n e[n,h])
Host does the tiny O(B) folds exactly in fp32: qk = (context@Wq)@Wk^T/8,
agg = (xe @ Wv) / S, out = (qc*query + agg) @ Wf summed over heads.

Performance structure (CoreSim cost model, ~9.2us/core vs 20.3us baseline):
  - x is shipped once in EACH layout (x_T for u, x_nat-with-ones for xe),
    both fp8 SBUF images pre-arranged in DRAM so every DMA is a
    max-contiguous 2D copy of a graph PAIR (above the 500ns floor).
  - Per-graph-exact tile counts: the multiset of tile counts is equalized
    across cores (slight zero-padding) so all cores run one program.
  - 3 DMA queues (SP, Pool, Act). Act's queue is poisoned for feeding
    (consumers of its exps wait on earlier queue DMAs' DGE-delayed
    completions), so Act carries only the one-time Exp table load, two
    late-block xt pairs (plus the fp8 qk appendix), the exps, and the
    final output DMA. The xt wave leads both SP/Pool streams (via a
    logical-time ladder) so exps fire as early as possible; xns stream
    behind them in block order.
  - xe uses fp8 DoubleRow matmuls; the two contracted node-tiles are
    (i, i+4) so the dual weight rows sit a multiple of 16B apart (ISA
    restriction); leftovers run as plain matmuls.
  - PSUM evacuation on DVE + late pairs on ACT; results are packed at
    partition offsets 0/32/64/96 so ONE tiny output DMA (500ns floor)
    drains everything.
Pad nodes: x_T pad columns are zero -> u=0 -> e=exp(-1), but their
x_nat rows AND ones-column entries are zero, so they contribute nothing
to xe or S. No host-side pad correction needed.
"""

import sys

if "/opt/trn_rl_repo" not in sys.path:
    sys.path.insert(0, "/opt/trn_rl_repo")

from contextlib import ExitStack

import numpy as np
import ml_dtypes

import concourse.bass as bass
import concourse.tile as tile
from concourse import bacc, mybir
from concourse.bass_utils import run_bass_kernel_spmd

N_CORES = 8
H = 8          # heads
DV = 64        # head dim
DE = 124       # output embedding dim
F = 128        # node feature dim (DE + 4)
FW = F + 1     # features + ones column (S rides along as column 128)
B = 128        # graphs
GPC = B // N_CORES  # graphs per core
F8 = ml_dtypes.float8_e4m3

_CACHE = {}


def _build(slot_T, W):
    """Build the Bass module. slot_T[s] = tile count of slot s; W[s] = exact
    x_T image width (max node count over cores). Tail u-matmul tiles read
    past W into the neighbouring image: the garbage scores are harmless
    because x_nat pad rows (and their ones-column) are zero."""
    slot_T = list(slot_T)
    W = list(W)
    nc = bacc.Bacc(None, target_bir_lowering=False)
    fp8 = mybir.dt.float8e4
    fp16 = mybir.dt.float16
    f32 = mybir.dt.float32
    AFT = mybir.ActivationFunctionType

    NPAIR = GPC // 2
    # per-slot image widths and flat per-pair DMA offsets
    xn_w = [t * FW for t in slot_T]
    # qkt (fp8) rides inside pair NP-2's xt DMA: insert its 128 columns
    # right after slot GPC-3's image in the flat XT layout
    QCOLS = GPC * H
    xt_w2 = list(W)
    xt_w2[GPC - 3] += QCOLS
    xt_off = np.concatenate([[0], np.cumsum(xt_w2)]).astype(int)
    xn_off = np.concatenate([[0], np.cumsum(xn_w)]).astype(int)
    # final zero pad lets the last pair's tail tile spill safely
    XT = nc.dram_tensor(
        "XT", [128, int(xt_off[-1]) + 256], fp8, kind="ExternalInput"
    )
    XN = nc.dram_tensor("XN", [128, int(xn_off[-1])], fp8, kind="ExternalInput")
    OCH = (NPAIR + 3) // 4          # column chunks of the packed output
    XEALL = nc.dram_tensor(
        "XEALL", [128, OCH * 2 * FW], fp16, kind="ExternalOutput"
    )

    with tile.TileContext(nc) as tc, ExitStack() as ctx:
        const = ctx.enter_context(tc.tile_pool(name="const", bufs=1))
        xtp = ctx.enter_context(tc.tile_pool(name="xt", bufs=GPC // 2))
        xnp = ctx.enter_context(tc.tile_pool(name="xn", bufs=GPC // 2))
        epool = ctx.enter_context(tc.tile_pool(name="e", bufs=4))
        outp = ctx.enter_context(tc.tile_pool(name="outp", bufs=1))
        ps_u = ctx.enter_context(
            tc.tile_pool(name="ps_u", bufs=2, space=bass.MemorySpace.PSUM)
        )
        ps_xe = ctx.enter_context(
            tc.tile_pool(name="ps_xe", bufs=6, space=bass.MemorySpace.PSUM)
        )

        NB = GPC // 4           # blocks of 4 graphs
        NP = GPC // 2           # graph pairs

        biasm1 = const.tile([128, 1], f32)
        nc.vector.memset(biasm1[:], -1.0)
        xeall = outp.tile([128, OCH * 2 * FW], fp16)
        nc.vector.memset(xeall[:], 0.0)

        # Consumers of an exp wait on Act's monotonic sem counter, so any
        # Act DMA issued before that exp delays the exp's consumers until
        # the DMA *completes* (proc + ~1.7us DGE delay). Act therefore
        # carries only a few EARLY xts from LATE blocks (completions land
        # before those blocks' xns are ready anyway); everything else
        # feeds through SP/Pool. Early blocks' xts lead both streams so
        # the first exps fire early.
        xt_q = {0: nc.sync, 1: nc.gpsimd, 2: nc.sync, 3: nc.gpsimd,
                4: nc.sync, 5: nc.gpsimd, 6: nc.scalar, 7: nc.scalar}
        xn_q = {0: nc.gpsimd, 1: nc.sync, 2: nc.gpsimd, 3: nc.sync,
                4: nc.gpsimd, 5: nc.sync, 6: nc.gpsimd, 7: nc.sync}

        xt_t, xn_t = [None] * GPC, [None] * GPC
        qkt = None
        # xts lead every queue: they feed u->exp whose consumers cascade
        with tc.high_priority():
            for gp in range(NP):
                s0, s1 = 2 * gp, 2 * gp + 1
                w0, w1 = xt_w2[s0], xt_w2[s1]
                # s1's tail tile may spill past its image: transfer the
                # spill bytes too (they belong to the next pair / pad)
                ext = max(0, 128 * slot_T[s1] - w1)
                t_xt = xtp.tile([128, w0 + w1 + ext], fp8, tag="xt",
                                name=f"xt{gp}")
                xt_q[gp % 8].dma_start(
                    t_xt[:],
                    XT[:, int(xt_off[s0]) : int(xt_off[s0]) + w0 + w1 + ext],
                )
                xt_t[s0] = t_xt[:, 0 : 128 * slot_T[s0]]
                xt_t[s1] = t_xt[:, w0 : w0 + 128 * slot_T[s1]]
                if s1 == GPC - 3:
                    qkt = t_xt[:, w0 + W[s1] : w0 + w1]
        # logical-time ladder: xns schedule strictly after the xt wave so
        # exps (gated by xts through the DGE delay) fire as early as
        # possible; xns then stream in block order
        xn_rank = {gp: gp for gp in range(NP)}
        for gp in range(NP):
            s0, s1 = 2 * gp, 2 * gp + 1
            w0, w1 = xn_w[s0], xn_w[s1]
            t_xn = xnp.tile([128, w0 + w1 + 3 * FW], fp8, tag="xn",
                            name=f"xn{gp}")
            with tc.tile_wait_until(0.002 + 0.0002 * xn_rank[gp]):
                xn_q[gp % 8].dma_start(
                    t_xn[:, 0 : w0 + w1],
                    XN[:, int(xn_off[s0]) : int(xn_off[s1 + 1])],
                )
            # views extend past each slot's span (into the neighbour slot /
            # the tile's pad columns) so the wide DoubleRow rearrange slices
            # stay within bounds; actual matmul reads never leave the slot
            xn_t[s0] = t_xn[:, 0 : w0 + w1]
            xn_t[s1] = t_xn[:, w0 : w0 + w1 + 3 * FW]

        e_t = [None] * NB

        # e-column offset of each slot within its block's e tile
        e_off = [0] * GPC
        for blk0 in range(NB):
            acc = 0
            for j in range(4):
                e_off[4 * blk0 + j] = acc
                acc += slot_T[4 * blk0 + j] * H

        def blk_cols(blk):
            return sum(slot_T[4 * blk + j] for j in range(4)) * H

        def emit_block(blk):
            u_ps = ps_u.tile([128, blk_cols(blk)], f32, tag="u",
                             name=f"u{blk}")
            for j in range(4):
                g = 4 * blk + j
                for t in range(slot_T[g]):
                    c = e_off[g] + t * H
                    nc.tensor.matmul(
                        u_ps[:, c : c + H],
                        xt_t[g][:, bass.ts(t, F)],
                        qkt[:, bass.ts(g, H)],
                        start=True,
                        stop=True,
                    )
            e_sb = epool.tile([128, blk_cols(blk) + 8 * H], fp8, tag="e",
                              name=f"e{blk}")
            # zero the pad columns: the dual-fp8 weight load may touch a few
            # bytes past the last slot's e values
            nc.vector.memset(e_sb[:, blk_cols(blk) :], 0.0)
            nc.scalar.activation(e_sb[:, 0 : blk_cols(blk)], u_ps[:],
                                 AFT.Exp, bias=biasm1[:])
            e_t[blk] = e_sb

        def emit_xe(blk, pair):
            p = 2 * blk + pair
            xe_ps = ps_xe.tile([H, 2 * FW], f32, tag="xe",
                               name=f"xe{blk}_{pair}")
            e_sb = e_t[blk]
            for half in range(2):
                j = 2 * pair + half
                g = 4 * blk + j
                Tg = slot_T[g]
                dst = xe_ps[:, half * FW : (half + 1) * FW]
                # dual-fp8 Ldweights requires the two k-tiles' weights a
                # multiple of 16B apart: pair tile i with i+m (m even)
                m = 4 if Tg >= 5 else (2 if Tg >= 3 else 1)
                prs = list(range(max(0, Tg - m))) if Tg > m else []
                sgl = list(range(max(0, Tg - m), min(m, Tg)))
                nops = len(prs) + len(sgl)
                k = 0
                for i in prs:
                    c = e_off[g] + i * H
                    nc.tensor.matmul(
                        dst,
                        e_sb[:, c : c + 2 * m * H]
                        .rearrange("p (two r) -> p two r", two=2)[:, :, 0:H],
                        xn_t[g][:, i * FW : (i + 2 * m) * FW]
                        .rearrange("p (two r) -> p two r", two=2)[:, :, 0:FW],
                        start=(k == 0),
                        stop=(k == nops - 1),
                        perf_mode=mybir.MatmulPerfMode.DoubleRow,
                    )
                    k += 1
                for t in sgl:
                    c = e_off[g] + t * H
                    nc.tensor.matmul(
                        dst,
                        e_sb[:, c : c + H],
                        xn_t[g][:, t * FW : (t + 1) * FW],
                        start=(k == 0),
                        stop=(k == nops - 1),
                    )
                    k += 1
            # pack pair p's [8, 2*FW] block at partition offset 32*(p%4),
            # column chunk p//4 of the output tile
            r0 = 32 * (p % 4)
            c0 = (p // 4) * 2 * FW
            dst = xeall[r0 : r0 + H, c0 : c0 + 2 * FW]
            # copies on DVE; drain-critical last pairs on ACT (idle
            # once the exps are done). GPSIMD cannot touch PSUM on hw.
            if p >= NPAIR - 2:
                nc.scalar.copy(dst, xe_ps[:])
            else:
                nc.vector.tensor_copy(dst, xe_ps[:])

        # software pipeline: a block's xe work is emitted right after the
        # NEXT block's scores, so a late xt never blocks earlier xes on
        # the PE stream while exp has a block of lead time
        emit_block(0)
        emit_block(1)
        emit_xe(0, 0)
        emit_xe(0, 1)
        emit_block(2)
        emit_xe(1, 0)
        emit_xe(1, 1)
        emit_block(3)
        for blk, pair in ((2, 0), (2, 1), (3, 0), (3, 1)):
            emit_xe(blk, pair)

        # one tiny output DMA: per-partition bytes are small thanks to
        # the partition packing, so this costs only the 500ns floor
        nc.scalar.dma_start(XEALL[:], xeall[:])

    nc.compile()
    return nc


def _get(slot_T, W):
    key = (tuple(slot_T), tuple(W))
    if key not in _CACHE:
        _CACHE[key] = _build(slot_T, W)
    return _CACHE[key]


def _prepare(x, batch, context, Wq, Wk):
    """Host-side shard prep with exact per-graph tile counts.

    Every core must run the same compiled program, so the multiset of tile
    counts is equalized across cores: for each tile-count value, the number
    of graphs is rounded down to a multiple of N_CORES by promoting the
    excess (zero-padding them one extra tile). Graphs are then assigned to
    (core, slot) with slots sorted by descending tile count.
    Returns (in_maps, slot_T, W, query, gmap); gmap[core][slot] = graph,
    W[slot] = exact x_T image width (max count over cores).
    """
    counts = np.bincount(batch, minlength=B).astype(np.int64)
    starts = np.zeros(B + 1, np.int64)
    np.cumsum(counts, out=starts[1:])

    Tg = np.maximum(1, np.ceil(counts / 128.0)).astype(np.int64)
    # promote excess graphs per tile-count value (smallest values first);
    # spill at the top value cascades upward until every count divides
    t = int(Tg.min())
    while t <= int(Tg.max()):
        idx = np.where(Tg == t)[0]
        r = len(idx) % N_CORES
        if r:
            Tg[idx[-r:]] += 1    # zero-pad these graphs one extra tile
        t += 1

    # slots sorted by descending T; same layout on every core
    order = np.argsort(-Tg, kind="stable")      # graph ids, desc T
    slot_T = [int(Tg[order[s * N_CORES]]) for s in range(GPC)]
    gmap = [[int(order[s * N_CORES + c]) for s in range(GPC)]
            for c in range(N_CORES)]

    query = np.einsum("bc,hcv->hbv", context, Wq).astype(np.float32)  # [H,B,Dv]
    qk = np.einsum("hbv,hev->hbe", query, Wk).astype(np.float32)      # [H,B,F]
    qk8 = (qk / 8.0).astype(np.float16)

    x8 = x.astype(F8)
    # exact x_T widths: max count over cores per slot; within each pair put
    # the slot with the LARGER padding saving first (its tail-tile spill is
    # free, the second slot's spill costs transferred bytes)
    W = [max(int(counts[gmap[c][s]]) for c in range(N_CORES))
         for s in range(GPC)]
    for p in range(GPC // 2):
        a, b2 = 2 * p, 2 * p + 1
        if (128 * slot_T[a] - W[a]) < (128 * slot_T[b2] - W[b2]):
            slot_T[a], slot_T[b2] = slot_T[b2], slot_T[a]
            W[a], W[b2] = W[b2], W[a]
            for c in range(N_CORES):
                gmap[c][a], gmap[c][b2] = gmap[c][b2], gmap[c][a]
    xn_w = [t * FW for t in slot_T]
    # qkt (fp8) rides after slot GPC-3's x_T image (see _build)
    QCOLS = GPC * H
    xt_w2 = list(W)
    xt_w2[GPC - 3] += QCOLS
    xt_off = np.concatenate([[0], np.cumsum(xt_w2)]).astype(int)
    xn_off = np.concatenate([[0], np.cumsum(xn_w)]).astype(int)
    qk_col = int(xt_off[GPC - 3]) + W[GPC - 3]

    in_maps = []
    for c in range(N_CORES):
        XTc = np.zeros((128, int(xt_off[-1]) + 256), F8)
        XNc = np.zeros((128, int(xn_off[-1])), F8)
        for s in range(GPC):
            b = gmap[c][s]
            T = slot_T[s]
            G = 128 * T
            n0, n1 = int(starts[b]), int(starts[b + 1])
            cnt = n1 - n0
            seg = x8[n0:n1]                       # [cnt, 128]
            XTc[:, int(xt_off[s]) : int(xt_off[s]) + cnt] = seg.T
            nat = np.zeros((G, FW), F8)
            nat[:cnt, :F] = seg
            nat[:cnt, F] = np.float32(1.0)
            XNc[:, int(xn_off[s]) : int(xn_off[s + 1])] = (
                nat.reshape(T, 128, FW).transpose(1, 0, 2).reshape(128, T * FW)
            )
            XTc[:, qk_col + s * H : qk_col + (s + 1) * H] = qk8[:, b, :].T.astype(F8)
        in_maps.append({"XT": XTc, "XN": XNc})
    return in_maps, slot_T, W, query, gmap


def kernel(**inputs):
    x = np.asarray(inputs["x"], np.float32)
    batch = np.asarray(inputs["batch"]).astype(np.int64)
    context = np.asarray(inputs["context"], np.float32)
    Wq = np.asarray(inputs["Wq"], np.float32)
    Wk = np.asarray(inputs["Wk"], np.float32)
    Wv = np.asarray(inputs["Wv"], np.float32)
    qc = float(np.asarray(inputs["query_coef"]).reshape(-1)[0])
    Wf = np.asarray(inputs["Wf"], np.float32)

    in_maps, slot_T, W, query, gmap = _prepare(x, batch, context, Wq, Wk)

    nc = _get(slot_T, W)

    def run_device():
        res = run_bass_kernel_spmd(nc, in_maps, core_ids=list(range(N_CORES)))
        XE = np.zeros((H, B, F), np.float32)
        S = np.zeros((H, B), np.float32)
        for c in range(N_CORES):
            packed = np.asarray(res.results[c]["XEALL"], np.float32)
            for p in range(GPC // 2):
                r0, c0 = 32 * (p % 4), (p // 4) * 2 * FW
                blk = packed[r0 : r0 + H, c0 : c0 + 2 * FW].reshape(H, 2, FW)
                for half in range(2):
                    b = gmap[c][2 * p + half]
                    XE[:, b, :] = blk[:, half, :F]
                    S[:, b] = blk[:, half, F]
        return XE, S

    XE, S = run_device()
    if not (np.isfinite(XE).all() and np.isfinite(S).all()):
        XE, S = run_device()    # retry once on a transient bad run

    Y = np.einsum("hbe,hev->hbv", XE, Wv.astype(np.float32))
    agg = Y / (S[..., None] + 1e-16)
    hbv = qc * query + agg
    out = np.einsum("hbv,ve->be", hbv, Wf)
    return out.astype(np.float32)


# revision 57
# speedup vs baseline: 1.0027x; 1.0027x over previous
"""Trainium2 Bass kernel for nn_AttentionLayerDecoder (sparse segment attention).

Math (reference, with edge_index unused):
  query[h,b,v] = context[b,:] @ Wq[h]                      # [H,B,Dv]
  u[h,n]      = (x[n,:] @ Wk[h]) . query[h,batch[n],:] / sqrt(Dv)
  a[h,n]      = segment_softmax(u) over nodes of graph batch[n]
  agg[h,b,v]  = sum_{n in b} a[h,n] * (x[n,:] @ Wv[h])
  out[b,:]    = sum_h (qc*query[h,b,:] + agg[h,b,:]) @ Wf

Device computes, per graph g (sharded 16 whole graphs per core):
  u[n,h]  = x[n,:] . qk[g,h,:]        (PE, lhsT = x_T tile, rhs = qk)
  e[n,h]  = exp(u - 1)                (ACT, batched 4 graphs, fp8 out;
                                       -1 bias keeps e in fp8e4m3 range)
  xe[h,:] = sum_n e[n,h] * [x[n,:],1] (PE fp8 DoubleRow, ones column
                                       makes col 128 = S = sum_n e[n,h])
Host does the tiny O(B) folds exactly in fp32: qk = (context@Wq)@Wk^T/8,
agg = (xe @ Wv) / S, out = (qc*query + agg) @ Wf summed over heads.

Performance structure (CoreSim cost model, ~9.2us/core vs 20.3us baseline):
  - x is shipped once in EACH layout (x_T for u, x_nat-with-ones for xe),
    both fp8 SBUF images pre-arranged in DRAM so every DMA is a
    max-contiguous 2D copy of a graph PAIR (above the 500ns floor).
  - Per-graph-exact tile counts: the multiset of tile counts is equalized
    across cores (slight zero-padding) so all cores run one program.
  - 3 DMA queues (SP, Pool, Act). Act's queue is poisoned for feeding
    (consumers of its exps wait on earlier queue DMAs' DGE-delayed
    completions), so Act carries only the one-time Exp table load, two
    late-block xt pairs (plus the fp8 qk appendix), the exps, and the
    final output DMA. The xt wave leads both SP/Pool streams (via a
    logical-time ladder) so exps fire as early as possible; xns stream
    behind them in block order.
  - xe uses fp8 DoubleRow matmuls; the two contracted node-tiles are
    (i, i+4) so the dual weight rows sit a multiple of 16B apart (ISA
    restriction); leftovers run as plain matmuls.
  - PSUM evacuation on DVE + late pairs on ACT; results are packed at
    partition offsets 0/32/64/96 so ONE tiny output DMA (500ns floor)
    drains everything.
Pad nodes: x_T pad columns are zero -> u=0 -> e=exp(-1), but their
x_nat rows AND ones-column entries are zero, so they contribute nothing
to xe or S. No host-side pad correction needed.
"""

import sys

if "/opt/trn_rl_repo" not in sys.path:
    sys.path.insert(0, "/opt/trn_rl_repo")

from contextlib import ExitStack

import numpy as np
import ml_dtypes

import concourse.bass as bass
import concourse.tile as tile
from concourse import bacc, mybir
from concourse.bass_utils import run_bass_kernel_spmd

N_CORES = 8
H = 8          # heads
DV = 64        # head dim
DE = 124       # output embedding dim
F = 128        # node feature dim (DE + 4)
FW = F + 1     # features + ones column (S rides along as column 128)
B = 128        # graphs
GPC = B // N_CORES  # graphs per core
F8 = ml_dtypes.float8_e4m3

_CACHE = {}


def _build(slot_T, W):
    """Build the Bass module. slot_T[s] = tile count of slot s; W[s] = exact
    x_T image width (max node count over cores). Tail u-matmul tiles read
    past W into the neighbouring image: the garbage scores are harmless
    because x_nat pad rows (and their ones-column) are zero."""
    slot_T = list(slot_T)
    W = list(W)
    nc = bacc.Bacc(None, target_bir_lowering=False)
    fp8 = mybir.dt.float8e4
    fp16 = mybir.dt.float16
    f32 = mybir.dt.float32
    AFT = mybir.ActivationFunctionType

    NPAIR = GPC // 2
    # per-slot image widths and flat per-pair DMA offsets
    xn_w = [t * FW for t in slot_T]
    # qkt (fp8) rides inside pair NP-2's xt DMA: insert its 128 columns
    # right after slot GPC-3's image in the flat XT layout
    QCOLS = GPC * H
    xt_w2 = list(W)
    xt_w2[GPC - 3] += QCOLS
    xt_off = np.concatenate([[0], np.cumsum(xt_w2)]).astype(int)
    xn_off = np.concatenate([[0], np.cumsum(xn_w)]).astype(int)
    # final zero pad lets the last pair's tail tile spill safely
    XT = nc.dram_tensor(
        "XT", [128, int(xt_off[-1]) + 256], fp8, kind="ExternalInput"
    )
    XN = nc.dram_tensor("XN", [128, int(xn_off[-1])], fp8, kind="ExternalInput")
    OCH = (NPAIR + 3) // 4          # column chunks of the packed output
    XEALL = nc.dram_tensor(
        "XEALL", [128, OCH * 2 * FW], fp16, kind="ExternalOutput"
    )

    with tile.TileContext(nc) as tc, ExitStack() as ctx:
        const = ctx.enter_context(tc.tile_pool(name="const", bufs=1))
        xtp = ctx.enter_context(tc.tile_pool(name="xt", bufs=GPC // 2))
        xnp = ctx.enter_context(tc.tile_pool(name="xn", bufs=GPC // 2))
        epool = ctx.enter_context(tc.tile_pool(name="e", bufs=4))
        outp = ctx.enter_context(tc.tile_pool(name="outp", bufs=1))
        ps_u = ctx.enter_context(
            tc.tile_pool(name="ps_u", bufs=2, space=bass.MemorySpace.PSUM)
        )
        ps_xe = ctx.enter_context(
            tc.tile_pool(name="ps_xe", bufs=6, space=bass.MemorySpace.PSUM)
        )

        NB = GPC // 4           # blocks of 4 graphs
        NP = GPC // 2           # graph pairs

        biasm1 = const.tile([128, 1], f32)
        nc.vector.memset(biasm1[:], -1.0)
        xeall = outp.tile([128, OCH * 2 * FW], fp16)
        nc.vector.memset(xeall[:], 0.0)

        # Consumers of an exp wait on Act's monotonic sem counter, so any
        # Act DMA issued before that exp delays the exp's consumers until
        # the DMA *completes* (proc + ~1.7us DGE delay). Act therefore
        # carries only a few EARLY xts from LATE blocks (completions land
        # before those blocks' xns are ready anyway); everything else
        # feeds through SP/Pool. Early blocks' xts lead both streams so
        # the first exps fire early.
        xt_q = {0: nc.sync, 1: nc.gpsimd, 2: nc.sync, 3: nc.gpsimd,
                4: nc.sync, 5: nc.gpsimd, 6: nc.scalar, 7: nc.scalar}
        xn_q = {0: nc.gpsimd, 1: nc.sync, 2: nc.gpsimd, 3: nc.sync,
                4: nc.gpsimd, 5: nc.sync, 6: nc.gpsimd, 7: nc.sync}

        xt_t, xn_t = [None] * GPC, [None] * GPC
        qkt = None
        # xts lead every queue: they feed u->exp whose consumers cascade
        with tc.high_priority():
            for gp in range(NP):
                s0, s1 = 2 * gp, 2 * gp + 1
                w0, w1 = xt_w2[s0], xt_w2[s1]
                t_xt = xtp.tile([128, w0 + w1], fp8, tag="xt",
                                name=f"xt{gp}")
                xt_q[gp % 8].dma_start(
                    t_xt[:],
                    XT[:, int(xt_off[s0]) : int(xt_off[s0]) + w0 + w1],
                )
                xt_t[s0] = t_xt[:, 0:w0]
                xt_t[s1] = t_xt[:, w0 : w0 + W[s1]]
                if s1 == GPC - 3:
                    qkt = t_xt[:, w0 + W[s1] : w0 + w1]
        # logical-time ladder: xns schedule strictly after the xt wave so
        # exps (gated by xts through the DGE delay) fire as early as
        # possible; xns then stream in block order
        xn_rank = {gp: gp for gp in range(NP)}
        for gp in range(NP):
            s0, s1 = 2 * gp, 2 * gp + 1
            w0, w1 = xn_w[s0], xn_w[s1]
            t_xn = xnp.tile([128, w0 + w1 + 3 * FW], fp8, tag="xn",
                            name=f"xn{gp}")
            with tc.tile_wait_until(0.002 + 0.0002 * xn_rank[gp]):
                xn_q[gp % 8].dma_start(
                    t_xn[:, 0 : w0 + w1],
                    XN[:, int(xn_off[s0]) : int(xn_off[s1 + 1])],
                )
            # views extend past each slot's span (into the neighbour slot /
            # the tile's pad columns) so the wide DoubleRow rearrange slices
            # stay within bounds; actual matmul reads never leave the slot
            xn_t[s0] = t_xn[:, 0 : w0 + w1]
            xn_t[s1] = t_xn[:, w0 : w0 + w1 + 3 * FW]

        e_t = [None] * NB

        # e-column offset of each slot within its block's e tile
        e_off = [0] * GPC
        for blk0 in range(NB):
            acc = 0
            for j in range(4):
                e_off[4 * blk0 + j] = acc
                acc += slot_T[4 * blk0 + j] * H

        def blk_cols(blk):
            return sum(slot_T[4 * blk + j] for j in range(4)) * H

        def emit_block(blk):
            u_ps = ps_u.tile([128, blk_cols(blk)], f32, tag="u",
                             name=f"u{blk}")
            for j in range(4):
                g = 4 * blk + j
                Tg = slot_T[g]
                for t in range(Tg):
                    c = e_off[g] + t * H
                    # tail tile: shifted window over the last 128 valid
                    # columns (duplicated nodes are zeroed in x_nat)
                    x0 = t * 128 if t < Tg - 1 else max(0, W[g] - 128)
                    nc.tensor.matmul(
                        u_ps[:, c : c + H],
                        xt_t[g][:, x0 : x0 + min(128, W[g])],
                        qkt[:, bass.ts(g, H)],
                        start=True,
                        stop=True,
                    )
            e_sb = epool.tile([128, blk_cols(blk) + 8 * H], fp8, tag="e",
                              name=f"e{blk}")
            # zero the pad columns: the dual-fp8 weight load may touch a few
            # bytes past the last slot's e values
            nc.vector.memset(e_sb[:, blk_cols(blk) :], 0.0)
            nc.scalar.activation(e_sb[:, 0 : blk_cols(blk)], u_ps[:],
                                 AFT.Exp, bias=biasm1[:])
            e_t[blk] = e_sb

        def emit_xe(blk, pair):
            p = 2 * blk + pair
            xe_ps = ps_xe.tile([H, 2 * FW], f32, tag="xe",
                               name=f"xe{blk}_{pair}")
            e_sb = e_t[blk]
            for half in range(2):
                j = 2 * pair + half
                g = 4 * blk + j
                Tg = slot_T[g]
                dst = xe_ps[:, half * FW : (half + 1) * FW]
                # dual-fp8 Ldweights requires the two k-tiles' weights a
                # multiple of 16B apart: pair tile i with i+m (m even)
                m = 4 if Tg >= 5 else (2 if Tg >= 3 else 1)
                prs = list(range(max(0, Tg - m))) if Tg > m else []
                sgl = list(range(max(0, Tg - m), min(m, Tg)))
                nops = len(prs) + len(sgl)
                k = 0
                for i in prs:
                    c = e_off[g] + i * H
                    nc.tensor.matmul(
                        dst,
                        e_sb[:, c : c + 2 * m * H]
                        .rearrange("p (two r) -> p two r", two=2)[:, :, 0:H],
                        xn_t[g][:, i * FW : (i + 2 * m) * FW]
                        .rearrange("p (two r) -> p two r", two=2)[:, :, 0:FW],
                        start=(k == 0),
                        stop=(k == nops - 1),
                        perf_mode=mybir.MatmulPerfMode.DoubleRow,
                    )
                    k += 1
                for t in sgl:
                    c = e_off[g] + t * H
                    nc.tensor.matmul(
                        dst,
                        e_sb[:, c : c + H],
                        xn_t[g][:, t * FW : (t + 1) * FW],
                        start=(k == 0),
                        stop=(k == nops - 1),
                    )
                    k += 1
            # pack pair p's [8, 2*FW] block at partition offset 32*(p%4),
            # column chunk p//4 of the output tile
            r0 = 32 * (p % 4)
            c0 = (p // 4) * 2 * FW
            dst = xeall[r0 : r0 + H, c0 : c0 + 2 * FW]
            # copies on DVE; drain-critical last pairs on ACT (idle
            # once the exps are done). GPSIMD cannot touch PSUM on hw.
            if p >= NPAIR - 2:
                nc.scalar.copy(dst, xe_ps[:])
            else:
                nc.vector.tensor_copy(dst, xe_ps[:])

        # software pipeline: a block's xe work is emitted right after the
        # NEXT block's scores, so a late xt never blocks earlier xes on
        # the PE stream while exp has a block of lead time
        emit_block(0)
        emit_block(1)
        emit_xe(0, 0)
        emit_xe(0, 1)
        emit_block(2)
        emit_xe(1, 0)
        emit_xe(1, 1)
        emit_block(3)
        for blk, pair in ((2, 0), (2, 1), (3, 0), (3, 1)):
            emit_xe(blk, pair)

        # one tiny output DMA: per-partition bytes are small thanks to
        # the partition packing, so this costs only the 500ns floor
        nc.scalar.dma_start(XEALL[:], xeall[:])

    nc.compile()
    return nc


def _get(slot_T, W):
    key = (tuple(slot_T), tuple(W))
    if key not in _CACHE:
        _CACHE[key] = _build(slot_T, W)
    return _CACHE[key]


def _prepare(x, batch, context, Wq, Wk):
    """Host-side shard prep with exact per-graph tile counts.

    Every core must run the same compiled program, so the multiset of tile
    counts is equalized across cores: for each tile-count value, the number
    of graphs is rounded down to a multiple of N_CORES by promoting the
    excess (zero-padding them one extra tile). Graphs are then assigned to
    (core, slot) with slots sorted by descending tile count.
    Returns (in_maps, slot_T, W, query, gmap); gmap[core][slot] = graph,
    W[slot] = exact x_T image width (max count over cores).
    """
    counts = np.bincount(batch, minlength=B).astype(np.int64)
    starts = np.zeros(B + 1, np.int64)
    np.cumsum(counts, out=starts[1:])

    Tg = np.maximum(1, np.ceil(counts / 128.0)).astype(np.int64)
    # promote excess graphs per tile-count value (smallest values first);
    # spill at the top value cascades upward until every count divides
    t = int(Tg.min())
    while t <= int(Tg.max()):
        idx = np.where(Tg == t)[0]
        r = len(idx) % N_CORES
        if r:
            Tg[idx[-r:]] += 1    # zero-pad these graphs one extra tile
        t += 1

    # slots sorted by descending T; same layout on every core
    order = np.argsort(-Tg, kind="stable")      # graph ids, desc T
    slot_T = [int(Tg[order[s * N_CORES]]) for s in range(GPC)]
    gmap = [[int(order[s * N_CORES + c]) for s in range(GPC)]
            for c in range(N_CORES)]

    query = np.einsum("bc,hcv->hbv", context, Wq).astype(np.float32)  # [H,B,Dv]
    qk = np.einsum("hbv,hev->hbe", query, Wk).astype(np.float32)      # [H,B,F]
    qk8 = (qk / 8.0).astype(np.float16)

    x8 = x.astype(F8)
    # exact x_T widths: max count over cores per slot; within each pair put
    # the slot with the LARGER padding saving first (its tail-tile spill is
    # free, the second slot's spill costs transferred bytes)
    W = [max(int(counts[gmap[c][s]]) for c in range(N_CORES))
         for s in range(GPC)]
    for p in range(GPC // 2):
        a, b2 = 2 * p, 2 * p + 1
        if (128 * slot_T[a] - W[a]) < (128 * slot_T[b2] - W[b2]):
            slot_T[a], slot_T[b2] = slot_T[b2], slot_T[a]
            W[a], W[b2] = W[b2], W[a]
            for c in range(N_CORES):
                gmap[c][a], gmap[c][b2] = gmap[c][b2], gmap[c][a]
    xn_w = [t * FW for t in slot_T]
    # qkt (fp8) rides after slot GPC-3's x_T image (see _build)
    QCOLS = GPC * H
    xt_w2 = list(W)
    xt_w2[GPC - 3] += QCOLS
    xt_off = np.concatenate([[0], np.cumsum(xt_w2)]).astype(int)
    xn_off = np.concatenate([[0], np.cumsum(xn_w)]).astype(int)
    qk_col = int(xt_off[GPC - 3]) + W[GPC - 3]

    in_maps = []
    for c in range(N_CORES):
        XTc = np.zeros((128, int(xt_off[-1]) + 256), F8)
        XNc = np.zeros((128, int(xn_off[-1])), F8)
        for s in range(GPC):
            b = gmap[c][s]
            T = slot_T[s]
            G = 128 * T
            n0, n1 = int(starts[b]), int(starts[b + 1])
            cnt = n1 - n0
            seg = x8[n0:n1]                       # [cnt, 128]
            XTc[:, int(xt_off[s]) : int(xt_off[s]) + cnt] = seg.T
            nat = np.zeros((G, FW), F8)
            nat[:cnt, :F] = seg
            nat[:cnt, F] = np.float32(1.0)
            img = nat.reshape(T, 128, FW).copy()
            # tail tile = shifted window [W-128, W): rows below (T-1)*128
            # duplicate earlier tiles and must contribute nothing
            lo = max(0, W[s] - 128)
            tail = np.zeros((128, FW), F8)
            v0 = (T - 1) * 128          # first non-duplicated node
            if cnt > v0:
                tail[v0 - lo : cnt - lo] = nat[v0:cnt]
            img[T - 1] = tail
            XNc[:, int(xn_off[s]) : int(xn_off[s + 1])] = (
                img.transpose(1, 0, 2).reshape(128, T * FW)
            )
            XTc[:, qk_col + s * H : qk_col + (s + 1) * H] = qk8[:, b, :].T.astype(F8)
        in_maps.append({"XT": XTc, "XN": XNc})
    return in_maps, slot_T, W, query, gmap


def kernel(**inputs):
    x = np.asarray(inputs["x"], np.float32)
    batch = np.asarray(inputs["batch"]).astype(np.int64)
    context = np.asarray(inputs["context"], np.float32)
    Wq = np.asarray(inputs["Wq"], np.float32)
    Wk = np.asarray(inputs["Wk"], np.float32)
    Wv = np.asarray(inputs["Wv"], np.float32)
    qc = float(np.asarray(inputs["query_coef"]).reshape(-1)[0])
    Wf = np.asarray(inputs["Wf"], np.float32)

    in_maps, slot_T, W, query, gmap = _prepare(x, batch, context, Wq, Wk)

    nc = _get(slot_T, W)

    def run_device():
        res = run_bass_kernel_spmd(nc, in_maps, core_ids=list(range(N_CORES)))
        XE = np.zeros((H, B, F), np.float32)
        S = np.zeros((H, B), np.float32)
        for c in range(N_CORES):
            packed = np.asarray(res.results[c]["XEALL"], np.float32)
            for p in range(GPC // 2):
                r0, c0 = 32 * (p % 4), (p // 4) * 2 * FW
                blk = packed[r0 : r0 + H, c0 : c0 + 2 * FW].reshape(H, 2, FW)
                for half in range(2):
                    b = gmap[c][2 * p + half]
                    XE[:, b, :] = blk[:, half, :F]
                    S[:, b] = blk[:, half, F]
        return XE, S

    XE, S = run_device()
    if not (np.isfinite(XE).all() and np.isfinite(S).all()):
        XE, S = run_device()    # retry once on a transient bad run

    Y = np.einsum("hbe,hev->hbv", XE, Wv.astype(np.float32))
    agg = Y / (S[..., None] + 1e-16)
    hbv = qc * query + agg
    out = np.einsum("hbv,ve->be", hbv, Wf)
    return out.astype(np.float32)
